# revision 4
# baseline (speedup 1.0000x reference)
"""DiffGraphTransformerSBM Trainium2 kernel.

Data-parallel over batch across 8 NeuronCores (4 graphs per core), with a
fully transpose-free feature-major layout on each core:

  - activations kept feature-major: h[f, t] (features on partitions)
  - scores computed directly transposed  sT[k, q] = kT.T @ qT
  - softmax without max-subtraction (scores are O(1e-2) by construction)
  - GraphiT renormalization folded algebraically:
        a = softmax(s)*pe / (sum(softmax(s)*pe) + 1e-6)
          = e*pe / sum_k(e*(pe + 1e-6)),  e = exp(s)
    so pe is pre-biased by 1e-6 on the host and the denominator comes for
    free from an all-ones column appended to V in the o-matmul.
  - per-token scalars (1/D, LN mean/rstd) broadcast across partitions via
    rank-1 (K=1) outer-product matmuls on the PE.

Self-contained: hardcodes all shapes; only imports the concourse toolchain.
"""

import os
import sys

import numpy as np
import ml_dtypes

for _p in ("/opt/trn_rl_repo", "/root/.axon_site/_ro/trn_rl_repo"):
    if os.path.isdir(_p) and _p not in sys.path:
        sys.path.append(_p)

import concourse.bass as bass
import concourse.bacc as bacc
import concourse.mybir as mybir
import concourse.tile as tile
from concourse import bass_utils

F32 = mybir.dt.float32
BF16 = mybir.dt.bfloat16
AF = mybir.ActivationFunctionType
ALU = mybir.AluOpType
bf = ml_dtypes.bfloat16

# problem dims
B, N, IN, D, H, DH, DFF, L, C = 32, 512, 64, 512, 8, 64, 2048, 4, 6
NCORES = 8
G = B // NCORES          # graphs per core
FC = D // 128            # feature chunks
KC = N // 128            # key-token chunks per graph
FFC = DFF // 128         # ffn chunks
T = G * N                # tokens per core


def _build(flags):
    """Build the per-core Bass program. flags: dict of which biases/affines exist."""
    nc = bacc.Bacc("TRN2", target_bir_lowering=False, debug=False, enable_asserts=False)

    # ---- DRAM I/O ----
    d_xT = nc.dram_tensor("xT", [G, IN, N], BF16, kind="ExternalInput").ap()
    d_peT = nc.dram_tensor("peT", [G, N, N], BF16, kind="ExternalInput").ap()
    d_deg = nc.dram_tensor("deg", [G, N], F32, kind="ExternalInput").ap()
    d_wembT = nc.dram_tensor("wembT", [IN, D], BF16, kind="ExternalInput").ap()
    d_qkvT = nc.dram_tensor("qkvT", [L, D, 3 * D], BF16, kind="ExternalInput").ap()
    d_owT = nc.dram_tensor("owT", [L, D, D], BF16, kind="ExternalInput").ap()
    d_l1T = nc.dram_tensor("l1T", [L, D, DFF], BF16, kind="ExternalInput").ap()
    d_l2T = nc.dram_tensor("l2T", [L, DFF, D], BF16, kind="ExternalInput").ap()
    d_c1T = nc.dram_tensor("c1T", [D, D], BF16, kind="ExternalInput").ap()
    d_c2T = nc.dram_tensor("c2T", [D, C], BF16, kind="ExternalInput").ap()
    # bias blob: per-partition columns (fp32), layout computed on host
    nbias = flags["nbias_cols"]
    d_bias = None
    if nbias:
        d_bias = nc.dram_tensor("biasblob", [128, nbias], F32, kind="ExternalInput").ap()
    d_vb = None
    if flags["vb"]:
        d_vb = nc.dram_tensor("vbias", [L, 1, D], BF16, kind="ExternalInput").ap()
    d_c2b = None
    if flags["c2b"]:
        d_c2b = nc.dram_tensor("c2bias", [1, C], BF16, kind="ExternalInput").ap()
    d_out = nc.dram_tensor("out", [T, C], F32, kind="ExternalOutput").ap()

    with tile.TileContext(nc) as tc:
        with tc.tile_pool(name="persist", bufs=1) as pp, \
             tc.tile_pool(name="wq", bufs=1) as wq, \
             tc.tile_pool(name="wo", bufs=2) as wo, \
             tc.tile_pool(name="w1", bufs=1) as w1p, \
             tc.tile_pool(name="w2", bufs=1) as w2p, \
             tc.tile_pool(name="work", bufs=1) as wk, \
             tc.tile_pool(name="ps", bufs=1, space="PSUM") as ps:

            # ---- persistent loads ----
            pe_sb = pp.tile([128, G * KC, N], BF16)
            nc.sync.dma_start(out=pe_sb, in_=d_peT.rearrange("g (kc p) q -> p (g kc) q", p=128))
            h_sb = pp.tile([128, FC, T], F32)
            hbf = pp.tile([128, FC, T], BF16)
            degB = pp.tile([128, G, N], F32)
            for g in range(G):
                src = d_deg[g:g + 1, :]
                src_bc = bass.AP(tensor=src.tensor, offset=src.offset,
                                 ap=[[0, 128]] + list(src.ap[1:]))
                nc.sync.dma_start(out=degB[:, g, :], in_=src_bc)
            wemb_sb = pp.tile([IN, D], BF16)
            nc.sync.dma_start(out=wemb_sb, in_=d_wembT)
            c1T_sb = pp.tile([128, FC, D], BF16)
            nc.sync.dma_start(out=c1T_sb, in_=d_c1T.rearrange("(kc p) f -> p kc f", p=128))
            c2T_sb = pp.tile([128, FC, C], BF16)
            nc.sync.dma_start(out=c2T_sb, in_=d_c2T.rearrange("(kc p) f -> p kc f", p=128))
            xT_sb = pp.tile([IN, G, N], BF16)
            nc.sync.dma_start(out=xT_sb, in_=d_xT.rearrange("g i q -> i g q"))

            ones64 = pp.tile([1, 64], BF16)
            nc.vector.memset(ones64, 1.0)
            ones128r = pp.tile([1, 128], BF16)
            nc.vector.memset(ones128r, 1.0)
            onesrow = pp.tile([1, N], BF16)
            nc.vector.memset(onesrow, 1.0)
            invn = pp.tile([128, 1], BF16)
            nc.vector.memset(invn, 1.0 / D)
            epsw = pp.tile([1, 1], BF16)
            nc.vector.memset(epsw, 1e-5)
            ones128f = pp.tile([1, 128], F32)
            nc.vector.memset(ones128f, 1.0)

            bias_sb = None
            if nbias:
                bias_sb = pp.tile([128, nbias], F32)
                nc.sync.dma_start(out=bias_sb, in_=d_bias)
            vb_sb = None
            if flags["vb"]:
                vb_sb = pp.tile([L, 1, D], BF16)
                nc.sync.dma_start(out=vb_sb, in_=d_vb)
            c2b_sb = None
            if flags["c2b"]:
                c2b_sb = pp.tile([1, C], BF16)
                nc.sync.dma_start(out=c2b_sb, in_=d_c2b)

            bcol = flags["bias_cols"]  # dict name -> start col in bias blob

            def bias_ap(name, l, idx):
                return bias_sb[:, bcol[name] + l * {"qkvb": 8, "ob": 4, "l1b": 16,
                                                    "l2b": 4, "ln1s": 4, "ln1b": 4,
                                                    "ln2s": 4, "ln2b": 4, "c1b": 0}[name] + idx:
                               bcol[name] + l * {"qkvb": 8, "ob": 4, "l1b": 16, "l2b": 4,
                                                 "ln1s": 4, "ln1b": 4, "ln2s": 4,
                                                 "ln2b": 4, "c1b": 0}[name] + idx + 1]

            # ---- embedding: h0[f, t] = W_emb.T.T @ xT ----
            for g in range(G):
                gsl = slice(g * N, (g + 1) * N)
                for fc in range(FC):
                    e_ps = ps.tile([128, N], F32, tag="mm", bufs=4, name="emb_ps")
                    nc.tensor.matmul(e_ps, wemb_sb[:, fc * 128:(fc + 1) * 128],
                                     xT_sb[:, g, :], start=True, stop=True)
                    nc.scalar.activation(h_sb[:, fc, gsl], e_ps, AF.Copy)
                    nc.vector.tensor_copy(hbf[:, fc, gsl], e_ps)

            # ---- transformer layers ----
            for l in range(L):
                qkv_t = wq.tile([128, KC, 3 * D], BF16, name="qkv_t")
                nc.sync.dma_start(out=qkv_t, in_=d_qkvT[l].rearrange("(kc p) f -> p kc f", p=128))
                ow_t = wo.tile([128, KC, D], BF16, name="ow_t")
                nc.sync.dma_start(out=ow_t, in_=d_owT[l].rearrange("(kc p) f -> p kc f", p=128))
                l1_t = w1p.tile([128, KC, DFF], BF16, name="l1_t")
                nc.sync.dma_start(out=l1_t, in_=d_l1T[l].rearrange("(kc p) f -> p kc f", p=128))
                l2_t = w2p.tile([128, FFC, D], BF16, name="l2_t")
                nc.sync.dma_start(out=l2_t, in_=d_l2T[l].rearrange("(kc p) f -> p kc f", p=128))

                for g in range(G):
                    gsl = slice(g * N, (g + 1) * N)
                    # ---- QKV ----
                    qT = wk.tile([128, FC, N], BF16, name="qT")
                    kT = wk.tile([128, FC, N], BF16, name="kT")
                    for fc in range(2 * FC):  # 0..3 q, 4..7 k
                        qk_ps = ps.tile([128, N], F32, tag="mm", bufs=4, name="qk_ps")
                        for kc in range(KC):
                            nc.tensor.matmul(qk_ps, qkv_t[:, kc, fc * 128:(fc + 1) * 128],
                                             hbf[:, kc, gsl],
                                             start=(kc == 0), stop=(kc == KC - 1))
                        dst = qT[:, fc, :] if fc < FC else kT[:, fc - FC, :]
                        if flags["qkvb"]:
                            nc.scalar.activation(dst, qk_ps, AF.Copy,
                                                 bias=bias_ap("qkvb", l, fc))
                        else:
                            nc.scalar.activation(dst, qk_ps, AF.Copy)
                    v_sb = wk.tile([128, KC, H * 65], BF16, name="v_sb")
                    ones_cols = bass.AP(tensor=v_sb.tensor, offset=v_sb.offset + 64,
                                        ap=[v_sb.ap[0], [H * 65, KC], [65, H]])
                    nc.vector.memset(ones_cols, 1.0)
                    for tc4 in range(KC):
                        v_ps = ps.tile([128, D], F32, tag="mm", bufs=4, name="v_ps")
                        for kc in range(KC):
                            nc.tensor.matmul(v_ps, hbf[:, kc, g * N + tc4 * 128:
                                                         g * N + (tc4 + 1) * 128],
                                             qkv_t[:, kc, 2 * D:3 * D],
                                             start=(kc == 0), stop=(kc == KC - 1 and not flags["vb"]))
                        if flags["vb"]:
                            nc.tensor.matmul(v_ps, ones128r, vb_sb[l], start=False, stop=True)
                        grp_dst = bass.AP(tensor=v_sb.tensor,
                                          offset=v_sb.offset + tc4 * (H * 65),
                                          ap=[v_sb.ap[0], [65, H], [1, 64]])
                        nc.vector.tensor_copy(grp_dst, v_ps.rearrange("p (h d) -> p h d", h=H))

                    # ---- attention (2 heads per partition-tile group) ----
                    ou = wk.tile([128, FC, N], BF16, name="ou")
                    for grp in range(4):
                        o_ps = []
                        for j in range(2):
                            o_ps.append(ps.tile([128, N], F32, tag="mm", bufs=4,
                                                name=f"o_ps{j}"))
                        for kc in range(KC):
                            s_ps = ps.tile([128, 2 * N], F32, tag="sc", bufs=2, name="s_ps")
                            for j in range(2):
                                hh = grp * 2 + j
                                po = (hh % 2) * 64
                                pt = hh // 2
                                nc.tensor.matmul(
                                    s_ps[:, j * N:(j + 1) * N],
                                    kT[po:po + 64, pt, kc * 128:(kc + 1) * 128],
                                    qT[po:po + 64, pt, :], start=True, stop=True)
                            e_t = wk.tile([128, 2, N], BF16, bufs=2, name="e_t")
                            nc.scalar.activation(e_t.rearrange("p h q -> p (h q)"), s_ps, AF.Exp)
                            m_t = wk.tile([128, 2, N], BF16, bufs=2, name="m_t")
                            pe_slice = pe_sb[:, g * KC + kc, :]
                            pe_bc = bass.AP(tensor=pe_slice.tensor, offset=pe_slice.offset,
                                            ap=[pe_slice.ap[0], [0, 2]] + list(pe_slice.ap[1:]))
                            nc.vector.tensor_mul(m_t, e_t, pe_bc)
                            for j in range(2):
                                hh = grp * 2 + j
                                nc.tensor.matmul(o_ps[j][0:65, :],
                                                 v_sb[:, kc, hh * 65:hh * 65 + 65],
                                                 m_t[:, j, :],
                                                 start=(kc == 0), stop=(kc == KC - 1))
                        # normalization: G = deg / D broadcast over the 128 partitions
                        DB_ps = ps.tile([128, N], F32, tag="mm", bufs=4, name="DB_ps")
                        for j in range(2):
                            drow = wk.tile([1, N], BF16, bufs=2, name="drow")
                            nc.scalar.activation(drow, o_ps[j][64:65, :], AF.Copy)
                            nc.tensor.matmul(DB_ps[64 * j:64 * (j + 1), :], ones64, drow,
                                             start=True, stop=True,
                                             tile_position=(0, 64 * j))
                        rec = wk.tile([128, N], F32, bufs=1, name="rec")
                        nc.vector.reciprocal(rec, DB_ps)
                        nc.vector.tensor_mul(rec, rec, degB[:, g, :])
                        for j in range(2):
                            nc.vector.tensor_mul(ou[64 * j:64 * (j + 1), grp, :],
                                                 o_ps[j][0:64, :],
                                                 rec[64 * j:64 * (j + 1), :])

                    # ---- out-proj + residual -> h_sb (pre-LN u) ----
                    for fc in range(FC):
                        op_ps = ps.tile([128, N], F32, tag="mm", bufs=4, name="op_ps")
                        for dc in range(FC):
                            nc.tensor.matmul(op_ps, ow_t[:, dc, fc * 128:(fc + 1) * 128],
                                             ou[:, dc, :],
                                             start=(dc == 0), stop=(dc == FC - 1 and not flags["ob"]))
                        if flags["ob"]:
                            nc.tensor.matmul(op_ps, ones128r, _row_of(nc, wk, bias_sb, bcol, "ob", l, fc, N),
                                             start=False, stop=True)
                        nc.vector.tensor_add(h_sb[:, fc, gsl], h_sb[:, fc, gsl], op_ps)

                    _layernorm(nc, tc, wk, ps, flags, bias_sb, bcol, "ln1", l, g,
                               h_sb, hbf, invn, epsw, onesrow, ones128f)

                    # ---- FFN ----
                    x1 = wk.tile([128, FFC, N], BF16, name="x1")
                    for ffc in range(FFC):
                        f1_ps = ps.tile([128, N], F32, tag="mm", bufs=4, name="f1_ps")
                        for kc in range(KC):
                            nc.tensor.matmul(f1_ps, l1_t[:, kc, ffc * 128:(ffc + 1) * 128],
                                             hbf[:, kc, gsl],
                                             start=(kc == 0), stop=(kc == KC - 1))
                        if flags["l1b"]:
                            nc.scalar.activation(x1[:, ffc, :], f1_ps, AF.Relu,
                                                 bias=bias_ap("l1b", l, ffc))
                        else:
                            nc.scalar.activation(x1[:, ffc, :], f1_ps, AF.Relu)
                    for fc in range(FC):
                        f2_ps = ps.tile([128, N], F32, tag="mm", bufs=4, name="f2_ps")
                        for kc in range(FFC):
                            nc.tensor.matmul(f2_ps, l2_t[:, kc, fc * 128:(fc + 1) * 128],
                                             x1[:, kc, :],
                                             start=(kc == 0), stop=(kc == FFC - 1 and not flags["l2b"]))
                        if flags["l2b"]:
                            nc.tensor.matmul(f2_ps, ones128r, _row_of(nc, wk, bias_sb, bcol, "l2b", l, fc, N),
                                             start=False, stop=True)
                        nc.vector.tensor_add(h_sb[:, fc, gsl], h_sb[:, fc, gsl], f2_ps)

                    _layernorm(nc, tc, wk, ps, flags, bias_sb, bcol, "ln2", l, g,
                               h_sb, hbf, invn, epsw, onesrow, ones128f)

            # ---- classifier ----
            for g in range(G):
                gsl = slice(g * N, (g + 1) * N)
                xcls = wk.tile([128, FC, N], BF16, name="xcls")
                for fc in range(FC):
                    c1_ps = ps.tile([128, N], F32, tag="mm", bufs=4, name="c1_ps")
                    for kc in range(FC):
                        nc.tensor.matmul(c1_ps, c1T_sb[:, kc, fc * 128:(fc + 1) * 128],
                                         hbf[:, kc, gsl],
                                         start=(kc == 0), stop=(kc == FC - 1))
                    if flags["c1b"]:
                        nc.scalar.activation(xcls[:, fc, :], c1_ps, AF.Relu,
                                             bias=bias_sb[:, bcol["c1b"] + fc:bcol["c1b"] + fc + 1])
                    else:
                        nc.scalar.activation(xcls[:, fc, :], c1_ps, AF.Relu)
                outsb = wk.tile([128, KC, C], F32, bufs=2, name="outsb")
                for tc4 in range(KC):
                    c2_ps = ps.tile([128, C], F32, tag="mm", bufs=4, name="c2_ps")
                    for ic in range(FC):
                        nc.tensor.matmul(c2_ps, xcls[:, ic, tc4 * 128:(tc4 + 1) * 128],
                                         c2T_sb[:, ic, :],
                                         start=(ic == 0), stop=(ic == FC - 1 and not flags["c2b"]))
                    if flags["c2b"]:
                        nc.tensor.matmul(c2_ps, ones128r, c2b_sb, start=False, stop=True)
                    nc.vector.tensor_copy(outsb[:, tc4, :], c2_ps)
                for tc4 in range(KC):
                    nc.sync.dma_start(out=d_out[g * N + tc4 * 128:g * N + (tc4 + 1) * 128, :],
                                      in_=outsb[:, tc4, :])

    nc.compile()
    return nc


def _row_of(nc, wk, bias_sb, bcol, name, l, fc, n):
    """Materialize a [1, n] bf16 row from a per-partition bias column via ...

    Not used when biases are zero; build a broadcastable row by copying the
    bias column (128 partitions) is wrong-axis, so this path requires a
    host-provided row tensor instead. Kept unreachable for zero biases."""
    raise NotImplementedError("non-zero out_b/lin2_b not supported in this build")


def _layernorm(nc, tc, wk, ps, flags, bias_sb, bcol, which, l, g,
               h_sb, hbf, invn, epsw, onesrow, ones128f):
    """LayerNorm over features (partitions) for graph g, in place on h_sb,
    writing the bf16 cast into hbf. h_sb currently holds the pre-LN u."""
    N_ = N
    gsl = slice(g * N_, (g + 1) * N_)
    # bf16 cast of u for the stats matmuls (into hbf, overwriting old h)
    for fc in range(FC):
        nc.vector.tensor_copy(hbf[:, fc, gsl], h_sb[:, fc, gsl])
    # sums over features via PE; eps folded into the sum-of-squares chain
    ms_ps = ps.tile([1, N_], F32, tag="mm", bufs=4, name="ms_ps")
    sq_ps = ps.tile([1, N_], F32, tag="mm", bufs=4, name="sq_ps")
    for fc in range(FC):
        nc.tensor.matmul(ms_ps, invn, hbf[:, fc, gsl],
                         start=(fc == 0), stop=(fc == FC - 1))
    for fc in range(FC):
        u2 = wk.tile([128, N_], BF16, bufs=2, name="u2")
        nc.scalar.activation(u2, h_sb[:, fc, gsl], AF.Square)
        nc.tensor.matmul(sq_ps, invn, u2, start=(fc == 0), stop=False)
    nc.tensor.matmul(sq_ps, epsw, onesrow, start=False, stop=True)
    # row math at [1, N]: var = E[u^2]+eps-m^2, r = sqrt(1/var)
    mrow = wk.tile([1, N_], F32, bufs=2, name="mrow")
    nc.scalar.activation(mrow, ms_ps, AF.Copy)
    vrow = wk.tile([1, N_], F32, bufs=2, name="vrow")
    nc.vector.tensor_mul(vrow, mrow, mrow)
    nc.vector.tensor_sub(vrow, sq_ps, vrow)
    nc.vector.reciprocal(vrow, vrow)
    rrow = wk.tile([1, N_], F32, bufs=2, name="rrow")
    nc.scalar.activation(rrow, vrow, AF.Sqrt)
    # broadcast m and r across partitions with K=1 fp32 outer products
    MB_ps = ps.tile([128, N_], F32, tag="mm", bufs=4, name="MB_ps")
    RB_ps = ps.tile([128, N_], F32, tag="mm", bufs=4, name="RB_ps")
    nc.tensor.matmul(MB_ps, ones128f, mrow, start=True, stop=True)
    nc.tensor.matmul(RB_ps, ones128f, rrow, start=True, stop=True)
    # apply: h = (u - MB) * RB  (optionally * s + b)
    s_on = flags[which + "s"]
    b_on = flags[which + "b"]
    for fc in range(FC):
        tt = wk.tile([128, N_], F32, bufs=2, name="tt")
        nc.vector.tensor_sub(tt, h_sb[:, fc, gsl], MB_ps)
        if not s_on and not b_on:
            nc.vector.tensor_mul(h_sb[:, fc, gsl], tt, RB_ps)
        else:
            nc.vector.tensor_mul(tt, tt, RB_ps)
            sc = bias_sb[:, bcol[which + "s"] + l * 4 + fc:bcol[which + "s"] + l * 4 + fc + 1] \
                if s_on else 1.0
            bi = bias_sb[:, bcol[which + "b"] + l * 4 + fc:bcol[which + "b"] + l * 4 + fc + 1] \
                if b_on else 0.0
            nc.scalar.activation(h_sb[:, fc, gsl], tt, AF.Copy, bias=bi, scale=sc)
        nc.vector.tensor_copy(hbf[:, fc, gsl], h_sb[:, fc, gsl])


_CACHE = {}


def _prep_inputs(inputs):
    """Host-side preprocessing -> (flags, per-core in_maps)."""
    x = np.asarray(inputs["x"], np.float32)
    pe = np.asarray(inputs["pe"], np.float32)
    degree = np.asarray(inputs["degree"], np.float32)
    W_emb = np.asarray(inputs["W_emb"], np.float32)
    qkv_w = np.asarray(inputs["qkv_w"], np.float32)
    qkv_b = np.asarray(inputs["qkv_b"], np.float32)
    out_w = np.asarray(inputs["out_w"], np.float32)
    out_b = np.asarray(inputs["out_b"], np.float32)
    lin1_w = np.asarray(inputs["lin1_w"], np.float32)
    lin1_b = np.asarray(inputs["lin1_b"], np.float32)
    lin2_w = np.asarray(inputs["lin2_w"], np.float32)
    lin2_b = np.asarray(inputs["lin2_b"], np.float32)
    ln1_s = np.asarray(inputs["ln1_s"], np.float32)
    ln1_b = np.asarray(inputs["ln1_b"], np.float32)
    ln2_s = np.asarray(inputs["ln2_s"], np.float32)
    ln2_b = np.asarray(inputs["ln2_b"], np.float32)
    cls1_w = np.asarray(inputs["cls1_w"], np.float32)
    cls1_b = np.asarray(inputs["cls1_b"], np.float32)
    cls2_w = np.asarray(inputs["cls2_w"], np.float32)
    cls2_b = np.asarray(inputs["cls2_b"], np.float32)

    flags = {
        "qkvb": bool(np.any(qkv_b[:, :2 * D])),
        "vb": bool(np.any(qkv_b[:, 2 * D:])),
        "ob": bool(np.any(out_b)),
        "l1b": bool(np.any(lin1_b)),
        "l2b": bool(np.any(lin2_b)),
        "c1b": bool(np.any(cls1_b)),
        "c2b": bool(np.any(cls2_b)),
        "ln1s": bool(np.any(ln1_s != 1.0)),
        "ln1b": bool(np.any(ln1_b)),
        "ln2s": bool(np.any(ln2_s != 1.0)),
        "ln2b": bool(np.any(ln2_b)),
    }
    if flags["ob"] or flags["l2b"]:
        raise NotImplementedError("non-zero out_b/lin2_b not supported")

    # bias blob layout
    cols = {}
    ncols = 0

    def add(name, width):
        nonlocal ncols
        cols[name] = ncols
        ncols += width

    blob_parts = []
    if flags["qkvb"]:
        add("qkvb", L * 8)
        qb = qkv_b[:, :2 * D].reshape(L, 8, 128)
        qb = qb.copy()
        qb[:, :4, :] *= 0.125  # q-bias carries the 1/sqrt(dh) fold
        blob_parts.append(qb.transpose(2, 0, 1).reshape(128, L * 8))
    if flags["l1b"]:
        add("l1b", L * 16)
        blob_parts.append(lin1_b.reshape(L, 16, 128).transpose(2, 0, 1).reshape(128, L * 16))
    if flags["c1b"]:
        add("c1b", 4)
        blob_parts.append(cls1_b.reshape(4, 128).T)
    for nm, arr in (("ln1s", ln1_s), ("ln1b", ln1_b), ("ln2s", ln2_s), ("ln2b", ln2_b)):
        if flags[nm]:
            add(nm, L * 4)
            blob_parts.append(arr.reshape(L, 4, 128).transpose(2, 0, 1).reshape(128, L * 4))
    flags["bias_cols"] = cols
    flags["nbias_cols"] = ncols
    blob = np.concatenate(blob_parts, axis=1).astype(np.float32) if blob_parts else None

    qkvT = np.ascontiguousarray(qkv_w.transpose(0, 2, 1)).copy()
    qkvT[:, :, :D] *= 0.125
    shared = {
        "wembT": np.ascontiguousarray(W_emb.T).astype(bf),
        "qkvT": qkvT.astype(bf),
        "owT": np.ascontiguousarray(out_w.transpose(0, 2, 1)).astype(bf),
        "l1T": np.ascontiguousarray(lin1_w.transpose(0, 2, 1)).astype(bf),
        "l2T": np.ascontiguousarray(lin2_w.transpose(0, 2, 1)).astype(bf),
        "c1T": np.ascontiguousarray(cls1_w.T).astype(bf),
        "c2T": np.ascontiguousarray(cls2_w.T).astype(bf),
    }
    if blob is not None:
        shared["biasblob"] = blob
    if flags["vb"]:
        shared["vbias"] = qkv_b[:, 2 * D:].reshape(L, 1, D).astype(bf)
    if flags["c2b"]:
        shared["c2bias"] = cls2_b.reshape(1, C).astype(bf)

    peb = (pe + 1e-6).transpose(0, 2, 1)
    xT = x.transpose(0, 2, 1)
    in_maps = []
    for c in range(NCORES):
        gs = slice(c * G, (c + 1) * G)
        im = dict(shared)
        im["xT"] = np.ascontiguousarray(xT[gs]).astype(bf)
        im["peT"] = np.ascontiguousarray(peb[gs]).astype(bf)
        im["deg"] = np.ascontiguousarray(degree[gs]).astype(np.float32)
        in_maps.append(im)
    return flags, in_maps


def kernel(**inputs):
    flags, in_maps = _prep_inputs(inputs)
    key = tuple(sorted((k, v) for k, v in flags.items() if k not in ("bias_cols",)
                       and not isinstance(v, dict)))
    if key not in _CACHE:
        _CACHE[key] = _build(flags)
    nc = _CACHE[key]
    res = bass_utils.run_bass_kernel_spmd(nc, in_maps, list(range(NCORES)))
    out = np.concatenate([r["out"] for r in res.results], axis=0)
    return out.astype(np.float32)


def run_traced(inputs, tmpdir="/tmp/ntff_out"):
    """For test.py: run with NTFF tracing, return (output, results)."""
    import shutil
    flags, in_maps = _prep_inputs(inputs)
    nc = _build(flags)
    if tmpdir:
        shutil.rmtree(tmpdir, ignore_errors=True)
        os.makedirs(tmpdir, exist_ok=True)
    res = bass_utils.run_bass_kernel_spmd(nc, in_maps, list(range(NCORES)),
                                          trace=True, tmpdir=tmpdir)
    out = np.concatenate([r["out"] for r in res.results], axis=0)
    return out.astype(np.float32), res


# revision 10
# speedup vs baseline: 1.2468x; 1.2468x over previous
"""DiffGraphTransformerSBM Trainium2 kernel.

Data-parallel over batch across 8 NeuronCores (4 graphs per core), with a
fully transpose-free feature-major layout on each core:

  - activations kept feature-major: h[f, t] (features on partitions)
  - scores computed directly transposed  sT[k, q] = kT.T @ qT
  - softmax without max-subtraction (scores are O(1e-2) by construction)
  - GraphiT renormalization folded algebraically:
        a = softmax(s)*pe / (sum(softmax(s)*pe) + 1e-6)
          = e*pe / sum_k(e*(pe + 1e-6)),  e = exp(s)
    so pe is pre-biased by 1e-6 on the host and the denominator comes for
    free from an all-ones column appended to V in the o-matmul.
  - per-token scalars (1/D, LN mean/rstd) broadcast across partitions via
    rank-1 (K=1) outer-product matmuls on the PE.

Self-contained: hardcodes all shapes; only imports the concourse toolchain.
"""

import os
import sys

import numpy as np
import ml_dtypes

for _p in ("/opt/trn_rl_repo", "/root/.axon_site/_ro/trn_rl_repo"):
    if os.path.isdir(_p) and _p not in sys.path:
        sys.path.append(_p)

import concourse.bass as bass
import concourse.bacc as bacc
import concourse.mybir as mybir
import concourse.tile as tile
from concourse import bass_utils

F32 = mybir.dt.float32
BF16 = mybir.dt.bfloat16
AF = mybir.ActivationFunctionType
ALU = mybir.AluOpType
bf = ml_dtypes.bfloat16

# problem dims
B, N, IN, D, H, DH, DFF, L, C = 32, 512, 64, 512, 8, 64, 2048, 4, 6
NCORES = 8
G = B // NCORES          # graphs per core
FC = D // 128            # feature chunks
KC = N // 128            # key-token chunks per graph
FFC = DFF // 128         # ffn chunks
T = G * N                # tokens per core


def _build(flags):
    """Build the per-core Bass program. flags: dict of which biases/affines exist."""
    nc = bacc.Bacc("TRN2", target_bir_lowering=False, debug=False, enable_asserts=False)

    # ---- DRAM I/O ----
    d_xT = nc.dram_tensor("xT", [G, IN, N], BF16, kind="ExternalInput").ap()
    d_peT = nc.dram_tensor("peT", [G, N, N], BF16, kind="ExternalInput").ap()
    d_deg = nc.dram_tensor("deg", [G, N], F32, kind="ExternalInput").ap()
    d_wembT = nc.dram_tensor("wembT", [IN, D], BF16, kind="ExternalInput").ap()
    d_qkvT = nc.dram_tensor("qkvT", [L, D, 3 * D], BF16, kind="ExternalInput").ap()
    d_owT = nc.dram_tensor("owT", [L, D, D], BF16, kind="ExternalInput").ap()
    d_l1T = nc.dram_tensor("l1T", [L, D, DFF], BF16, kind="ExternalInput").ap()
    d_l2T = nc.dram_tensor("l2T", [L, DFF, D], BF16, kind="ExternalInput").ap()
    d_c1T = nc.dram_tensor("c1T", [D, D], BF16, kind="ExternalInput").ap()
    d_c2T = nc.dram_tensor("c2T", [D, C], BF16, kind="ExternalInput").ap()
    # bias blob: per-partition columns (fp32), layout computed on host
    nbias = flags["nbias_cols"]
    d_bias = None
    if nbias:
        d_bias = nc.dram_tensor("biasblob", [128, nbias], F32, kind="ExternalInput").ap()
    d_vb = None
    if flags["vb"]:
        d_vb = nc.dram_tensor("vbias", [L, 1, D], BF16, kind="ExternalInput").ap()
    d_c2b = None
    if flags["c2b"]:
        d_c2b = nc.dram_tensor("c2bias", [1, C], BF16, kind="ExternalInput").ap()
    d_out = nc.dram_tensor("out", [T, C], F32, kind="ExternalOutput").ap()

    with tile.TileContext(nc) as tc:
        with tc.tile_pool(name="persist", bufs=1) as pp, \
             tc.tile_pool(name="wq", bufs=1) as wq, \
             tc.tile_pool(name="wo", bufs=2) as wo, \
             tc.tile_pool(name="w1", bufs=1) as w1p, \
             tc.tile_pool(name="w2", bufs=1) as w2p, \
             tc.tile_pool(name="work", bufs=1) as wk, \
             tc.tile_pool(name="ps", bufs=1, space="PSUM") as ps:

            # ---- persistent loads ----
            pe_sb = pp.tile([128, G * KC, N], BF16)
            nc.sync.dma_start(out=pe_sb, in_=d_peT.rearrange("g (kc p) q -> p (g kc) q", p=128))
            h_sb = pp.tile([128, FC, T], BF16)
            hbf = h_sb
            degB = pp.tile([128, G, N], F32)
            for g in range(G):
                src = d_deg[g:g + 1, :]
                src_bc = bass.AP(tensor=src.tensor, offset=src.offset,
                                 ap=[[0, 128]] + list(src.ap[1:]))
                nc.sync.dma_start(out=degB[:, g, :], in_=src_bc)
            wemb_sb = pp.tile([IN, D], BF16)
            nc.sync.dma_start(out=wemb_sb, in_=d_wembT)
            c1T_sb = pp.tile([128, FC, D], BF16)
            nc.sync.dma_start(out=c1T_sb, in_=d_c1T.rearrange("(kc p) f -> p kc f", p=128))
            c2T_sb = pp.tile([128, FC, C], BF16)
            nc.sync.dma_start(out=c2T_sb, in_=d_c2T.rearrange("(kc p) f -> p kc f", p=128))
            xT_sb = pp.tile([IN, G, N], BF16)
            nc.sync.dma_start(out=xT_sb, in_=d_xT.rearrange("g i q -> i g q"))

            ones64 = pp.tile([1, 64], BF16)
            nc.vector.memset(ones64, 1.0)
            ones128r = pp.tile([1, 128], BF16)
            nc.vector.memset(ones128r, 1.0)
            onesrow = pp.tile([1, N], BF16)
            nc.vector.memset(onesrow, 1.0)
            invn = pp.tile([128, 1], BF16)
            nc.vector.memset(invn, 1.0 / D)
            epsw = pp.tile([1, 1], BF16)
            nc.vector.memset(epsw, 1e-5)
            ones128f = pp.tile([97, 128], F32)
            nc.vector.memset(ones128f, 1.0)

            bias_sb = None
            if nbias:
                bias_sb = pp.tile([128, nbias], F32)
                nc.sync.dma_start(out=bias_sb, in_=d_bias)
            vb_sb = None
            if flags["vb"]:
                vb_sb = pp.tile([L, 1, D], BF16)
                nc.sync.dma_start(out=vb_sb, in_=d_vb)
            c2b_sb = None
            if flags["c2b"]:
                c2b_sb = pp.tile([1, C], BF16)
                nc.sync.dma_start(out=c2b_sb, in_=d_c2b)

            bcol = flags["bias_cols"]  # dict name -> start col in bias blob

            def bias_ap(name, l, idx):
                return bias_sb[:, bcol[name] + l * {"qkvb": 8, "ob": 4, "l1b": 16,
                                                    "l2b": 4, "ln1s": 4, "ln1b": 4,
                                                    "ln2s": 4, "ln2b": 4, "c1b": 0}[name] + idx:
                               bcol[name] + l * {"qkvb": 8, "ob": 4, "l1b": 16, "l2b": 4,
                                                 "ln1s": 4, "ln1b": 4, "ln2s": 4,
                                                 "ln2b": 4, "c1b": 0}[name] + idx + 1]

            # ---- embedding: h0[f, t] = W_emb.T.T @ xT ----
            for g in range(G):
                gsl = slice(g * N, (g + 1) * N)
                for fc in range(FC):
                    e_ps = ps.tile([128, N], F32, tag="mm", bufs=4, name="emb_ps")
                    nc.tensor.matmul(e_ps, wemb_sb[:, fc * 128:(fc + 1) * 128],
                                     xT_sb[:, g, :], start=True, stop=True)
                    nc.scalar.activation(h_sb[:, fc, gsl], e_ps, AF.Copy)

            # ---- transformer layers ----
            for l in range(L):
                qkv_t = wq.tile([128, KC, 3 * D], BF16, name="qkv_t")
                nc.sync.dma_start(out=qkv_t, in_=d_qkvT[l].rearrange("(kc p) f -> p kc f", p=128))
                ow_t = wo.tile([128, KC, D], BF16, name="ow_t")
                nc.sync.dma_start(out=ow_t, in_=d_owT[l].rearrange("(kc p) f -> p kc f", p=128))
                l1_t = w1p.tile([128, KC, DFF], BF16, name="l1_t")
                nc.sync.dma_start(out=l1_t, in_=d_l1T[l].rearrange("(kc p) f -> p kc f", p=128))
                l2_t = w2p.tile([128, FFC, D], BF16, name="l2_t")
                nc.sync.dma_start(out=l2_t, in_=d_l2T[l].rearrange("(kc p) f -> p kc f", p=128))

                # ---- phase 1: QKV + attention + out-proj + residual (all graphs) ----
                for g in range(G):
                    gsl = slice(g * N, (g + 1) * N)
                    # ---- QKV ----
                    qT = wk.tile([128, FC, N], BF16, bufs=2, name="qT")
                    kT = wk.tile([128, FC, N], BF16, bufs=2, name="kT")
                    for fc in range(2 * FC):  # 0..3 q, 4..7 k
                        qk_ps = ps.tile([128, N], F32, tag="mm", bufs=4, name="qk_ps")
                        for kc in range(KC):
                            nc.tensor.matmul(qk_ps, qkv_t[:, kc, fc * 128:(fc + 1) * 128],
                                             hbf[:, kc, gsl],
                                             start=(kc == 0), stop=(kc == KC - 1))
                        dst = qT[:, fc, :] if fc < FC else kT[:, fc - FC, :]
                        if flags["qkvb"]:
                            nc.scalar.activation(dst, qk_ps, AF.Copy,
                                                 bias=bias_ap("qkvb", l, fc))
                        else:
                            nc.scalar.activation(dst, qk_ps, AF.Copy)
                    v_sb = wk.tile([128, KC, H * 65], BF16, bufs=2, name="v_sb")
                    ones_cols = bass.AP(tensor=v_sb.tensor, offset=v_sb.offset + 64,
                                        ap=[v_sb.ap[0], [H * 65, KC], [65, H]])
                    nc.vector.memset(ones_cols, 1.0)
                    for tc4 in range(KC):
                        v_ps = ps.tile([128, D], F32, tag="mm", bufs=4, name="v_ps")
                        for kc in range(KC):
                            nc.tensor.matmul(v_ps, hbf[:, kc, g * N + tc4 * 128:
                                                         g * N + (tc4 + 1) * 128],
                                             qkv_t[:, kc, 2 * D:3 * D],
                                             start=(kc == 0), stop=(kc == KC - 1 and not flags["vb"]))
                        if flags["vb"]:
                            nc.tensor.matmul(v_ps, ones128r, vb_sb[l], start=False, stop=True)
                        grp_dst = bass.AP(tensor=v_sb.tensor,
                                          offset=v_sb.offset + tc4 * (H * 65),
                                          ap=[v_sb.ap[0], [65, H], [1, 64]])
                        nc.vector.tensor_copy(grp_dst, v_ps.rearrange("p (h d) -> p h d", h=H))

                    # ---- attention (2 heads per partition-tile group) ----
                    ou = wk.tile([128, FC, N], BF16, bufs=2, name="ou")
                    for grp in range(4):
                        o_ps = []
                        for j in range(2):
                            o_ps.append(ps.tile([128, N], F32, tag="mm", bufs=4,
                                                name=f"o_ps{j}"))
                        for kc in range(KC):
                            s_ps = ps.tile([128, 2 * N], F32, tag="sc", bufs=2, name="s_ps")
                            for j in range(2):
                                hh = grp * 2 + j
                                po = (hh % 2) * 64
                                pt = hh // 2
                                nc.tensor.matmul(
                                    s_ps[:, j * N:(j + 1) * N],
                                    kT[po:po + 64, pt, kc * 128:(kc + 1) * 128],
                                    qT[po:po + 64, pt, :], start=True, stop=True)
                            e_t = wk.tile([128, 2, N], BF16, bufs=2, name="e_t")
                            nc.scalar.activation(e_t.rearrange("p h q -> p (h q)"), s_ps, AF.Exp)
                            m_t = wk.tile([128, 2, N], BF16, bufs=2, name="m_t")
                            pe_slice = pe_sb[:, g * KC + kc, :]
                            pe_bc = bass.AP(tensor=pe_slice.tensor, offset=pe_slice.offset,
                                            ap=[pe_slice.ap[0], [0, 2]] + list(pe_slice.ap[1:]))
                            nc.vector.tensor_mul(m_t, e_t, pe_bc)
                            for j in range(2):
                                hh = grp * 2 + j
                                nc.tensor.matmul(o_ps[j][0:65, :],
                                                 v_sb[:, kc, hh * 65:hh * 65 + 65],
                                                 m_t[:, j, :],
                                                 start=(kc == 0), stop=(kc == KC - 1))
                        # normalization: G = deg / D broadcast over the 128 partitions
                        DB_ps = ps.tile([128, N], F32, tag="mm", bufs=4, name="DB_ps")
                        for j in range(2):
                            drow = wk.tile([1, N], BF16, bufs=2, name="drow")
                            nc.scalar.activation(drow, o_ps[j][64:65, :], AF.Copy)
                            nc.tensor.matmul(DB_ps[64 * j:64 * (j + 1), :], ones64, drow,
                                             start=True, stop=True,
                                             tile_position=(0, 64 * j))
                        DB_sb = wk.tile([128, N], F32, bufs=1, name="DB_sb")
                        nc.scalar.activation(DB_sb, DB_ps, AF.Copy)
                        rec = wk.tile([128, N], F32, bufs=2, name="rec")
                        nc.vector.reciprocal_approx_fast(out=rec, in_=DB_sb)
                        nc.vector.tensor_mul(rec, rec, degB[:, g, :])
                        for j in range(2):
                            nc.vector.tensor_mul(ou[64 * j:64 * (j + 1), grp, :],
                                                 o_ps[j][0:64, :],
                                                 rec[64 * j:64 * (j + 1), :])

                    # ---- out-proj + residual -> h_sb (pre-LN u) ----
                    for fc in range(FC):
                        op_ps = ps.tile([128, N], F32, tag="mm", bufs=4, name="op_ps")
                        for dc in range(FC):
                            nc.tensor.matmul(op_ps, ow_t[:, dc, fc * 128:(fc + 1) * 128],
                                             ou[:, dc, :],
                                             start=(dc == 0), stop=(dc == FC - 1 and not flags["ob"]))
                        if flags["ob"]:
                            nc.tensor.matmul(op_ps, ones128r, _row_of(nc, wk, bias_sb, bcol, "ob", l, fc, N),
                                             start=False, stop=True)
                        nc.vector.tensor_add(h_sb[:, fc, gsl], h_sb[:, fc, gsl], op_ps)

                # ---- phase 2: LN1 (all graphs; one ACT table switch) ----
                for g in range(G):
                    _layernorm(nc, tc, wk, ps, flags, bias_sb, bcol, "ln1", l, g,
                               h_sb, hbf, invn, epsw, onesrow, ones128f)

                # ---- phase 3: FFN (all graphs) ----
                for g in range(G):
                    gsl = slice(g * N, (g + 1) * N)
                    x1 = wk.tile([128, FFC, N], BF16, bufs=2, name="x1")
                    for ffc in range(FFC):
                        f1_ps = ps.tile([128, N], F32, tag="mm", bufs=4, name="f1_ps")
                        for kc in range(KC):
                            nc.tensor.matmul(f1_ps, l1_t[:, kc, ffc * 128:(ffc + 1) * 128],
                                             hbf[:, kc, gsl],
                                             start=(kc == 0), stop=(kc == KC - 1))
                        if flags["l1b"]:
                            nc.scalar.activation(x1[:, ffc, :], f1_ps, AF.Relu,
                                                 bias=bias_ap("l1b", l, ffc))
                        else:
                            nc.scalar.activation(x1[:, ffc, :], f1_ps, AF.Relu)
                    for fc in range(FC):
                        f2_ps = ps.tile([128, N], F32, tag="mm", bufs=4, name="f2_ps")
                        for kc in range(FFC):
                            nc.tensor.matmul(f2_ps, l2_t[:, kc, fc * 128:(fc + 1) * 128],
                                             x1[:, kc, :],
                                             start=(kc == 0), stop=(kc == FFC - 1 and not flags["l2b"]))
                        if flags["l2b"]:
                            nc.tensor.matmul(f2_ps, ones128r, _row_of(nc, wk, bias_sb, bcol, "l2b", l, fc, N),
                                             start=False, stop=True)
                        nc.vector.tensor_add(h_sb[:, fc, gsl], h_sb[:, fc, gsl], f2_ps)

                # ---- phase 4: LN2 (all graphs) ----
                for g in range(G):
                    _layernorm(nc, tc, wk, ps, flags, bias_sb, bcol, "ln2", l, g,
                               h_sb, hbf, invn, epsw, onesrow, ones128f)

            # ---- classifier ----
            for g in range(G):
                gsl = slice(g * N, (g + 1) * N)
                xcls = wk.tile([128, FC, N], BF16, name="xcls")
                for fc in range(FC):
                    c1_ps = ps.tile([128, N], F32, tag="mm", bufs=4, name="c1_ps")
                    for kc in range(FC):
                        nc.tensor.matmul(c1_ps, c1T_sb[:, kc, fc * 128:(fc + 1) * 128],
                                         hbf[:, kc, gsl],
                                         start=(kc == 0), stop=(kc == FC - 1))
                    if flags["c1b"]:
                        nc.scalar.activation(xcls[:, fc, :], c1_ps, AF.Relu,
                                             bias=bias_sb[:, bcol["c1b"] + fc:bcol["c1b"] + fc + 1])
                    else:
                        nc.scalar.activation(xcls[:, fc, :], c1_ps, AF.Relu)
                outsb = wk.tile([128, KC, C], F32, bufs=2, name="outsb")
                for tc4 in range(KC):
                    c2_ps = ps.tile([128, C], F32, tag="mm", bufs=4, name="c2_ps")
                    for ic in range(FC):
                        nc.tensor.matmul(c2_ps, xcls[:, ic, tc4 * 128:(tc4 + 1) * 128],
                                         c2T_sb[:, ic, :],
                                         start=(ic == 0), stop=(ic == FC - 1 and not flags["c2b"]))
                    if flags["c2b"]:
                        nc.tensor.matmul(c2_ps, ones128r, c2b_sb, start=False, stop=True)
                    nc.vector.tensor_copy(outsb[:, tc4, :], c2_ps)
                for tc4 in range(KC):
                    nc.sync.dma_start(out=d_out[g * N + tc4 * 128:g * N + (tc4 + 1) * 128, :],
                                      in_=outsb[:, tc4, :])

    nc.compile()
    return nc


def _row_of(nc, wk, bias_sb, bcol, name, l, fc, n):
    """Materialize a [1, n] bf16 row from a per-partition bias column via ...

    Not used when biases are zero; build a broadcastable row by copying the
    bias column (128 partitions) is wrong-axis, so this path requires a
    host-provided row tensor instead. Kept unreachable for zero biases."""
    raise NotImplementedError("non-zero out_b/lin2_b not supported in this build")


def _layernorm(nc, tc, wk, ps, flags, bias_sb, bcol, which, l, g,
               h_sb, hbf, invn, epsw, onesrow, ones128f):
    """LayerNorm over features (partitions) for graph g, in place on h_sb,
    writing the bf16 cast into hbf. h_sb currently holds the pre-LN u."""
    N_ = N
    gsl = slice(g * N_, (g + 1) * N_)
    # sums over features via PE; eps folded into the sum-of-squares chain
    ms_ps = ps.tile([1, N_], F32, tag="mm", bufs=4, name="ms_ps")
    sq_ps = ps.tile([1, N_], F32, tag="mm", bufs=4, name="sq_ps")
    for fc in range(FC):
        nc.tensor.matmul(ms_ps, invn, hbf[:, fc, gsl],
                         start=(fc == 0), stop=(fc == FC - 1))
    for fc in range(FC):
        u2 = wk.tile([128, N_], BF16, bufs=2, name="u2")
        nc.scalar.activation(u2, h_sb[:, fc, gsl], AF.Square)
        nc.tensor.matmul(sq_ps, invn, u2, start=(fc == 0), stop=False)
    nc.tensor.matmul(sq_ps, epsw, onesrow, start=False, stop=True)
    # row math packed into one [128, N] tile: p0=m, p32=m^2/var
    rows = wk.tile([128, N_], F32, bufs=2, name="rows")
    nc.scalar.activation(rows[0:1, :], ms_ps, AF.Copy)
    nc.vector.tensor_mul(rows[32:33, :], rows[0:1, :], rows[0:1, :])
    nc.vector.tensor_sub(rows[32:33, :], sq_ps, rows[32:33, :])
    # broadcast m and var across partitions with K=1 fp32 outer products
    MB_ps = ps.tile([128, N_], F32, tag="mm", bufs=4, name="MB_ps")
    VB_ps = ps.tile([128, N_], F32, tag="mm", bufs=4, name="VB_ps")
    nc.tensor.matmul(MB_ps, ones128f[0:1, :], rows[0:1, :], start=True, stop=True)
    nc.tensor.matmul(VB_ps, ones128f[32:33, :], rows[32:33, :], start=True, stop=True)
    # wide 1/var (custom DVE, SBUF source) then wide sqrt -> rB in SBUF
    VB_sb = wk.tile([128, N_], F32, bufs=1, name="VB_sb")
    nc.scalar.activation(VB_sb, VB_ps, AF.Copy)
    RVB = wk.tile([128, N_], F32, bufs=1, name="RVB")
    nc.vector.reciprocal_approx_fast(out=RVB, in_=VB_sb)
    rB = wk.tile([128, N_], F32, bufs=2, name="rB")
    nc.scalar.activation(rB, RVB, AF.Sqrt)
    # apply: h = (u - MB) * rB  (optionally * s + b)
    s_on = flags[which + "s"]
    b_on = flags[which + "b"]
    for fc in range(FC):
        tt = wk.tile([128, N_], F32, bufs=2, name="tt")
        nc.vector.tensor_sub(tt, h_sb[:, fc, gsl], MB_ps)
        if not s_on and not b_on:
            nc.vector.tensor_mul(h_sb[:, fc, gsl], tt, rB)
        else:
            nc.vector.tensor_mul(tt, tt, rB)
            sc = bias_sb[:, bcol[which + "s"] + l * 4 + fc:bcol[which + "s"] + l * 4 + fc + 1] \
                if s_on else 1.0
            bi = bias_sb[:, bcol[which + "b"] + l * 4 + fc:bcol[which + "b"] + l * 4 + fc + 1] \
                if b_on else 0.0
            nc.scalar.activation(h_sb[:, fc, gsl], tt, AF.Copy, bias=bi, scale=sc)


_CACHE = {}


def _prep_inputs(inputs):
    """Host-side preprocessing -> (flags, per-core in_maps)."""
    x = np.asarray(inputs["x"], np.float32)
    pe = np.asarray(inputs["pe"], np.float32)
    degree = np.asarray(inputs["degree"], np.float32)
    W_emb = np.asarray(inputs["W_emb"], np.float32)
    qkv_w = np.asarray(inputs["qkv_w"], np.float32)
    qkv_b = np.asarray(inputs["qkv_b"], np.float32)
    out_w = np.asarray(inputs["out_w"], np.float32)
    out_b = np.asarray(inputs["out_b"], np.float32)
    lin1_w = np.asarray(inputs["lin1_w"], np.float32)
    lin1_b = np.asarray(inputs["lin1_b"], np.float32)
    lin2_w = np.asarray(inputs["lin2_w"], np.float32)
    lin2_b = np.asarray(inputs["lin2_b"], np.float32)
    ln1_s = np.asarray(inputs["ln1_s"], np.float32)
    ln1_b = np.asarray(inputs["ln1_b"], np.float32)
    ln2_s = np.asarray(inputs["ln2_s"], np.float32)
    ln2_b = np.asarray(inputs["ln2_b"], np.float32)
    cls1_w = np.asarray(inputs["cls1_w"], np.float32)
    cls1_b = np.asarray(inputs["cls1_b"], np.float32)
    cls2_w = np.asarray(inputs["cls2_w"], np.float32)
    cls2_b = np.asarray(inputs["cls2_b"], np.float32)

    flags = {
        "qkvb": bool(np.any(qkv_b[:, :2 * D])),
        "vb": bool(np.any(qkv_b[:, 2 * D:])),
        "ob": bool(np.any(out_b)),
        "l1b": bool(np.any(lin1_b)),
        "l2b": bool(np.any(lin2_b)),
        "c1b": bool(np.any(cls1_b)),
        "c2b": bool(np.any(cls2_b)),
        "ln1s": bool(np.any(ln1_s != 1.0)),
        "ln1b": bool(np.any(ln1_b)),
        "ln2s": bool(np.any(ln2_s != 1.0)),
        "ln2b": bool(np.any(ln2_b)),
    }
    if flags["ob"] or flags["l2b"]:
        raise NotImplementedError("non-zero out_b/lin2_b not supported")

    # bias blob layout
    cols = {}
    ncols = 0

    def add(name, width):
        nonlocal ncols
        cols[name] = ncols
        ncols += width

    blob_parts = []
    if flags["qkvb"]:
        add("qkvb", L * 8)
        qb = qkv_b[:, :2 * D].reshape(L, 8, 128)
        qb = qb.copy()
        qb[:, :4, :] *= 0.125  # q-bias carries the 1/sqrt(dh) fold
        blob_parts.append(qb.transpose(2, 0, 1).reshape(128, L * 8))
    if flags["l1b"]:
        add("l1b", L * 16)
        blob_parts.append(lin1_b.reshape(L, 16, 128).transpose(2, 0, 1).reshape(128, L * 16))
    if flags["c1b"]:
        add("c1b", 4)
        blob_parts.append(cls1_b.reshape(4, 128).T)
    for nm, arr in (("ln1s", ln1_s), ("ln1b", ln1_b), ("ln2s", ln2_s), ("ln2b", ln2_b)):
        if flags[nm]:
            add(nm, L * 4)
            blob_parts.append(arr.reshape(L, 4, 128).transpose(2, 0, 1).reshape(128, L * 4))
    flags["bias_cols"] = cols
    flags["nbias_cols"] = ncols
    blob = np.concatenate(blob_parts, axis=1).astype(np.float32) if blob_parts else None

    qkvT = np.ascontiguousarray(qkv_w.transpose(0, 2, 1)).copy()
    qkvT[:, :, :D] *= 0.125
    shared = {
        "wembT": np.ascontiguousarray(W_emb.T).astype(bf),
        "qkvT": qkvT.astype(bf),
        "owT": np.ascontiguousarray(out_w.transpose(0, 2, 1)).astype(bf),
        "l1T": np.ascontiguousarray(lin1_w.transpose(0, 2, 1)).astype(bf),
        "l2T": np.ascontiguousarray(lin2_w.transpose(0, 2, 1)).astype(bf),
        "c1T": np.ascontiguousarray(cls1_w.T).astype(bf),
        "c2T": np.ascontiguousarray(cls2_w.T).astype(bf),
    }
    if blob is not None:
        shared["biasblob"] = blob
    if flags["vb"]:
        shared["vbias"] = qkv_b[:, 2 * D:].reshape(L, 1, D).astype(bf)
    if flags["c2b"]:
        shared["c2bias"] = cls2_b.reshape(1, C).astype(bf)

    peb = (pe + 1e-6).transpose(0, 2, 1)
    xT = x.transpose(0, 2, 1)
    in_maps = []
    for c in range(NCORES):
        gs = slice(c * G, (c + 1) * G)
        im = dict(shared)
        im["xT"] = np.ascontiguousarray(xT[gs]).astype(bf)
        im["peT"] = np.ascontiguousarray(peb[gs]).astype(bf)
        im["deg"] = np.ascontiguousarray(degree[gs]).astype(np.float32)
        in_maps.append(im)
    return flags, in_maps


def kernel(**inputs):
    flags, in_maps = _prep_inputs(inputs)
    key = tuple(sorted((k, v) for k, v in flags.items() if k not in ("bias_cols",)
                       and not isinstance(v, dict)))
    if key not in _CACHE:
        _CACHE[key] = _build(flags)
    nc = _CACHE[key]
    res = bass_utils.run_bass_kernel_spmd(nc, in_maps, list(range(NCORES)))
    out = np.concatenate([r["out"] for r in res.results], axis=0)
    return out.astype(np.float32)


def run_traced(inputs, tmpdir="/tmp/ntff_out"):
    """For test.py: run with NTFF tracing, return (output, results)."""
    import shutil
    flags, in_maps = _prep_inputs(inputs)
    nc = _build(flags)
    if tmpdir:
        shutil.rmtree(tmpdir, ignore_errors=True)
        os.makedirs(tmpdir, exist_ok=True)
    res = bass_utils.run_bass_kernel_spmd(nc, in_maps, list(range(NCORES)),
                                          trace=True, tmpdir=tmpdir)
    out = np.concatenate([r["out"] for r in res.results], axis=0)
    return out.astype(np.float32), res


# revision 12
# speedup vs baseline: 1.3824x; 1.1088x over previous
"""DiffGraphTransformerSBM Trainium2 kernel.

Data-parallel over batch across 8 NeuronCores (4 graphs per core), with a
fully transpose-free feature-major layout on each core:

  - activations kept feature-major: h[f, t] (features on partitions)
  - scores computed directly transposed  sT[k, q] = kT.T @ qT
  - softmax without max-subtraction (scores are O(1e-2) by construction)
  - GraphiT renormalization folded algebraically:
        a = softmax(s)*pe / (sum(softmax(s)*pe) + 1e-6)
          = e*pe / sum_k(e*(pe + 1e-6)),  e = exp(s)
    so pe is pre-biased by 1e-6 on the host and the denominator comes for
    free from an all-ones column appended to V in the o-matmul.
  - per-token scalars (1/D, LN mean/rstd) broadcast across partitions via
    rank-1 (K=1) outer-product matmuls on the PE.

Self-contained: hardcodes all shapes; only imports the concourse toolchain.
"""

import os
import sys

import numpy as np
import ml_dtypes

for _p in ("/opt/trn_rl_repo", "/root/.axon_site/_ro/trn_rl_repo"):
    if os.path.isdir(_p) and _p not in sys.path:
        sys.path.append(_p)

import concourse.bass as bass
import concourse.bacc as bacc
import concourse.mybir as mybir
import concourse.tile as tile
from concourse import bass_utils

F32 = mybir.dt.float32
BF16 = mybir.dt.bfloat16
AF = mybir.ActivationFunctionType
ALU = mybir.AluOpType
bf = ml_dtypes.bfloat16

# problem dims
B, N, IN, D, H, DH, DFF, L, C = 32, 512, 64, 512, 8, 64, 2048, 4, 6
NCORES = 8
G = B // NCORES          # graphs per core
FC = D // 128            # feature chunks
KC = N // 128            # key-token chunks per graph
FFC = DFF // 128         # ffn chunks
T = G * N                # tokens per core


def _build(flags):
    """Build the per-core Bass program. flags: dict of which biases/affines exist."""
    nc = bacc.Bacc("TRN2", target_bir_lowering=False, debug=False, enable_asserts=False)

    # ---- DRAM I/O ----
    d_xT = nc.dram_tensor("xT", [G, IN, N], BF16, kind="ExternalInput").ap()
    d_peT = nc.dram_tensor("peT", [G, N, N], BF16, kind="ExternalInput").ap()
    d_deg = nc.dram_tensor("deg", [G, N], F32, kind="ExternalInput").ap()
    d_wembT = nc.dram_tensor("wembT", [IN, D], BF16, kind="ExternalInput").ap()
    d_qkvT = nc.dram_tensor("qkvT", [L, D, 3 * D], BF16, kind="ExternalInput").ap()
    d_owT = nc.dram_tensor("owT", [L, D, D], BF16, kind="ExternalInput").ap()
    d_l1T = nc.dram_tensor("l1T", [L, D, DFF], BF16, kind="ExternalInput").ap()
    d_l2T = nc.dram_tensor("l2T", [L, DFF, D], BF16, kind="ExternalInput").ap()
    d_c1T = nc.dram_tensor("c1T", [D, D], BF16, kind="ExternalInput").ap()
    d_c2T = nc.dram_tensor("c2T", [D, C], BF16, kind="ExternalInput").ap()
    # bias blob: per-partition columns (fp32), layout computed on host
    nbias = flags["nbias_cols"]
    d_bias = None
    if nbias:
        d_bias = nc.dram_tensor("biasblob", [128, nbias], F32, kind="ExternalInput").ap()
    d_vb = None
    if flags["vb"]:
        d_vb = nc.dram_tensor("vbias", [L, 1, D], BF16, kind="ExternalInput").ap()
    d_c2b = None
    if flags["c2b"]:
        d_c2b = nc.dram_tensor("c2bias", [1, C], BF16, kind="ExternalInput").ap()
    d_out = nc.dram_tensor("out", [T, C], F32, kind="ExternalOutput").ap()

    with tile.TileContext(nc) as tc:
        with tc.tile_pool(name="persist", bufs=1) as pp, \
             tc.tile_pool(name="wq", bufs=1) as wq, \
             tc.tile_pool(name="wo", bufs=2) as wo, \
             tc.tile_pool(name="w1", bufs=1) as w1p, \
             tc.tile_pool(name="w2", bufs=1) as w2p, \
             tc.tile_pool(name="work", bufs=1) as wk, \
             tc.tile_pool(name="ps", bufs=1, space="PSUM") as ps:

            # ---- persistent loads ----
            pe_sb = pp.tile([128, G * KC, N], BF16)
            nc.sync.dma_start(out=pe_sb, in_=d_peT.rearrange("g (kc p) q -> p (g kc) q", p=128))
            h_sb = pp.tile([128, FC, T], BF16)
            hbf = h_sb
            degB = pp.tile([128, G, N], F32)
            for g in range(G):
                src = d_deg[g:g + 1, :]
                src_bc = bass.AP(tensor=src.tensor, offset=src.offset,
                                 ap=[[0, 128]] + list(src.ap[1:]))
                nc.sync.dma_start(out=degB[:, g, :], in_=src_bc)
            wemb_sb = pp.tile([IN, D], BF16)
            nc.sync.dma_start(out=wemb_sb, in_=d_wembT)
            c1T_sb = pp.tile([128, FC, D], BF16)
            nc.sync.dma_start(out=c1T_sb, in_=d_c1T.rearrange("(kc p) f -> p kc f", p=128))
            c2T_sb = pp.tile([128, FC, C], BF16)
            nc.sync.dma_start(out=c2T_sb, in_=d_c2T.rearrange("(kc p) f -> p kc f", p=128))
            xT_sb = pp.tile([IN, G, N], BF16)
            nc.sync.dma_start(out=xT_sb, in_=d_xT.rearrange("g i q -> i g q"))

            ones64 = pp.tile([1, 64], BF16)
            nc.vector.memset(ones64, 1.0)
            ones128r = pp.tile([1, 128], BF16)
            nc.vector.memset(ones128r, 1.0)
            onesrow = pp.tile([1, N], BF16)
            nc.vector.memset(onesrow, 1.0)
            invn = pp.tile([128, 1], BF16)
            nc.vector.memset(invn, 1.0 / D)
            epsw = pp.tile([1, 1], BF16)
            nc.vector.memset(epsw, 1e-5)
            ones128f = pp.tile([97, 128], F32)
            nc.vector.memset(ones128f, 1.0)

            bias_sb = None
            if nbias:
                bias_sb = pp.tile([128, nbias], F32)
                nc.sync.dma_start(out=bias_sb, in_=d_bias)
            vb_sb = None
            if flags["vb"]:
                vb_sb = pp.tile([L, 1, D], BF16)
                nc.sync.dma_start(out=vb_sb, in_=d_vb)
            c2b_sb = None
            if flags["c2b"]:
                c2b_sb = pp.tile([1, C], BF16)
                nc.sync.dma_start(out=c2b_sb, in_=d_c2b)

            bcol = flags["bias_cols"]  # dict name -> start col in bias blob

            def bias_ap(name, l, idx):
                return bias_sb[:, bcol[name] + l * {"qkvb": 8, "ob": 4, "l1b": 16,
                                                    "l2b": 4, "ln1s": 4, "ln1b": 4,
                                                    "ln2s": 4, "ln2b": 4, "c1b": 0}[name] + idx:
                               bcol[name] + l * {"qkvb": 8, "ob": 4, "l1b": 16, "l2b": 4,
                                                 "ln1s": 4, "ln1b": 4, "ln2s": 4,
                                                 "ln2b": 4, "c1b": 0}[name] + idx + 1]

            # ---- embedding: h0[f, t] = W_emb.T.T @ xT ----
            for g in range(G):
                gsl = slice(g * N, (g + 1) * N)
                for fc in range(FC):
                    e_ps = ps.tile([128, N], F32, tag="mm", bufs=4, name="emb_ps")
                    nc.tensor.matmul(e_ps, wemb_sb[:, fc * 128:(fc + 1) * 128],
                                     xT_sb[:, g, :], start=True, stop=True)
                    nc.scalar.activation(h_sb[:, fc, gsl], e_ps, AF.Copy)

            # ---- transformer layers ----
            for l in range(L):
                qkv_t = wq.tile([128, KC, 3 * D], BF16, name="qkv_t")
                nc.sync.dma_start(out=qkv_t, in_=d_qkvT[l].rearrange("(kc p) f -> p kc f", p=128))
                ow_t = wo.tile([128, KC, D], BF16, name="ow_t")
                nc.sync.dma_start(out=ow_t, in_=d_owT[l].rearrange("(kc p) f -> p kc f", p=128))
                l1_t = w1p.tile([128, KC, DFF], BF16, name="l1_t")
                nc.sync.dma_start(out=l1_t, in_=d_l1T[l].rearrange("(kc p) f -> p kc f", p=128))
                l2_t = w2p.tile([128, FFC, D], BF16, name="l2_t")
                nc.sync.dma_start(out=l2_t, in_=d_l2T[l].rearrange("(kc p) f -> p kc f", p=128))

                # ---- phase 1: QKV + attention + out-proj + residual (all graphs) ----
                for g in range(G):
                    gsl = slice(g * N, (g + 1) * N)
                    # ---- QKV ----
                    qT = wk.tile([128, FC, N], BF16, bufs=2, name="qT")
                    kT = wk.tile([128, FC, N], BF16, bufs=2, name="kT")
                    for fc in range(2 * FC):  # 0..3 q, 4..7 k
                        qk_ps = ps.tile([128, N], F32, tag="mm", bufs=4, name="qk_ps")
                        for kc in range(KC):
                            nc.tensor.matmul(qk_ps, qkv_t[:, kc, fc * 128:(fc + 1) * 128],
                                             hbf[:, kc, gsl],
                                             start=(kc == 0), stop=(kc == KC - 1))
                        dst = qT[:, fc, :] if fc < FC else kT[:, fc - FC, :]
                        if flags["qkvb"]:
                            nc.scalar.activation(dst, qk_ps, AF.Copy,
                                                 bias=bias_ap("qkvb", l, fc))
                        else:
                            nc.vector.tensor_copy(dst, qk_ps)
                    v_sb = wk.tile([128, KC, H * 65], BF16, bufs=2, name="v_sb")
                    ones_cols = bass.AP(tensor=v_sb.tensor, offset=v_sb.offset + 64,
                                        ap=[v_sb.ap[0], [H * 65, KC], [65, H]])
                    nc.vector.memset(ones_cols, 1.0)
                    for tc4 in range(KC):
                        v_ps = ps.tile([128, D], F32, tag="mm", bufs=4, name="v_ps")
                        for kc in range(KC):
                            nc.tensor.matmul(v_ps, hbf[:, kc, g * N + tc4 * 128:
                                                         g * N + (tc4 + 1) * 128],
                                             qkv_t[:, kc, 2 * D:3 * D],
                                             start=(kc == 0), stop=(kc == KC - 1 and not flags["vb"]))
                        if flags["vb"]:
                            nc.tensor.matmul(v_ps, ones128r, vb_sb[l], start=False, stop=True)
                        grp_dst = bass.AP(tensor=v_sb.tensor,
                                          offset=v_sb.offset + tc4 * (H * 65),
                                          ap=[v_sb.ap[0], [65, H], [1, 64]])
                        nc.vector.tensor_copy(grp_dst, v_ps.rearrange("p (h d) -> p h d", h=H))

                    # ---- attention (2 heads per partition-tile group) ----
                    ou = wk.tile([128, FC, N], BF16, bufs=2, name="ou")
                    for grp in range(4):
                        o_ps = []
                        for j in range(2):
                            o_ps.append(ps.tile([128, N], F32, tag="mm", bufs=4,
                                                name=f"o_ps{j}"))
                        for kc in range(KC):
                            s_ps = ps.tile([128, 2 * N], F32, tag="sc", bufs=2, name="s_ps")
                            for j in range(2):
                                hh = grp * 2 + j
                                po = (hh % 2) * 64
                                pt = hh // 2
                                nc.tensor.matmul(
                                    s_ps[:, j * N:(j + 1) * N],
                                    kT[po:po + 64, pt, kc * 128:(kc + 1) * 128],
                                    qT[po:po + 64, pt, :], start=True, stop=True)
                            e_t = wk.tile([128, 2, N], BF16, bufs=3, name="e_t")
                            nc.scalar.activation(e_t.rearrange("p h q -> p (h q)"), s_ps, AF.Exp)
                            m_t = wk.tile([128, 2, N], BF16, bufs=3, name="m_t")
                            pe_slice = pe_sb[:, g * KC + kc, :]
                            pe_bc = bass.AP(tensor=pe_slice.tensor, offset=pe_slice.offset,
                                            ap=[pe_slice.ap[0], [0, 2]] + list(pe_slice.ap[1:]))
                            nc.vector.tensor_mul(m_t, e_t, pe_bc)
                            for j in range(2):
                                hh = grp * 2 + j
                                nc.tensor.matmul(o_ps[j][0:65, :],
                                                 v_sb[:, kc, hh * 65:hh * 65 + 65],
                                                 m_t[:, j, :],
                                                 start=(kc == 0), stop=(kc == KC - 1))
                        # normalization: G = deg / D broadcast over the 128 partitions
                        DB_ps = ps.tile([128, N], F32, tag="mm", bufs=4, name="DB_ps")
                        for j in range(2):
                            drow = wk.tile([1, N], BF16, bufs=2, name="drow")
                            nc.scalar.activation(drow, o_ps[j][64:65, :], AF.Copy)
                            nc.tensor.matmul(DB_ps[64 * j:64 * (j + 1), :], ones64, drow,
                                             start=True, stop=True,
                                             tile_position=(0, 64 * j))
                        DB_sb = wk.tile([128, N], F32, bufs=1, name="DB_sb")
                        nc.vector.tensor_copy(DB_sb, DB_ps)
                        rec = wk.tile([128, N], F32, bufs=1, name="rec")
                        nc.vector.reciprocal_approx_fast(out=rec, in_=DB_sb)
                        nc.vector.tensor_mul(rec, rec, degB[:, g, :])
                        for j in range(2):
                            nc.vector.tensor_mul(ou[64 * j:64 * (j + 1), grp, :],
                                                 o_ps[j][0:64, :],
                                                 rec[64 * j:64 * (j + 1), :])

                    # ---- out-proj + residual -> h_sb (pre-LN u) ----
                    for fc in range(FC):
                        op_ps = ps.tile([128, N], F32, tag="mm", bufs=4, name="op_ps")
                        for dc in range(FC):
                            nc.tensor.matmul(op_ps, ow_t[:, dc, fc * 128:(fc + 1) * 128],
                                             ou[:, dc, :],
                                             start=(dc == 0), stop=(dc == FC - 1 and not flags["ob"]))
                        if flags["ob"]:
                            nc.tensor.matmul(op_ps, ones128r, _row_of(nc, wk, bias_sb, bcol, "ob", l, fc, N),
                                             start=False, stop=True)
                        nc.vector.tensor_add(h_sb[:, fc, gsl], h_sb[:, fc, gsl], op_ps)

                # ---- phase 2: LN1 (all graphs; one ACT table switch) ----
                for g in range(G):
                    _layernorm(nc, tc, wk, ps, flags, bias_sb, bcol, "ln1", l, g,
                               h_sb, hbf, invn, epsw, onesrow, ones128f)

                # ---- phase 3: FFN (all graphs) ----
                for g in range(G):
                    gsl = slice(g * N, (g + 1) * N)
                    x1 = wk.tile([128, FFC, N], BF16, bufs=2, name="x1")
                    for ffc in range(FFC):
                        f1_ps = ps.tile([128, N], F32, tag="mm", bufs=4, name="f1_ps")
                        for kc in range(KC):
                            nc.tensor.matmul(f1_ps, l1_t[:, kc, ffc * 128:(ffc + 1) * 128],
                                             hbf[:, kc, gsl],
                                             start=(kc == 0), stop=(kc == KC - 1))
                        if flags["l1b"]:
                            nc.scalar.activation(x1[:, ffc, :], f1_ps, AF.Relu,
                                                 bias=bias_ap("l1b", l, ffc))
                        elif ffc % 2 == 0:
                            nc.vector.tensor_scalar_max(x1[:, ffc, :], f1_ps, 0.0)
                        else:
                            nc.scalar.activation(x1[:, ffc, :], f1_ps, AF.Relu)
                    for fc in range(FC):
                        f2_ps = ps.tile([128, N], F32, tag="mm", bufs=4, name="f2_ps")
                        for kc in range(FFC):
                            nc.tensor.matmul(f2_ps, l2_t[:, kc, fc * 128:(fc + 1) * 128],
                                             x1[:, kc, :],
                                             start=(kc == 0), stop=(kc == FFC - 1 and not flags["l2b"]))
                        if flags["l2b"]:
                            nc.tensor.matmul(f2_ps, ones128r, _row_of(nc, wk, bias_sb, bcol, "l2b", l, fc, N),
                                             start=False, stop=True)
                        nc.vector.tensor_add(h_sb[:, fc, gsl], h_sb[:, fc, gsl], f2_ps)

                # ---- phase 4: LN2 (all graphs) ----
                for g in range(G):
                    _layernorm(nc, tc, wk, ps, flags, bias_sb, bcol, "ln2", l, g,
                               h_sb, hbf, invn, epsw, onesrow, ones128f)

            # ---- classifier ----
            for g in range(G):
                gsl = slice(g * N, (g + 1) * N)
                xcls = wk.tile([128, FC, N], BF16, name="xcls")
                for fc in range(FC):
                    c1_ps = ps.tile([128, N], F32, tag="mm", bufs=4, name="c1_ps")
                    for kc in range(FC):
                        nc.tensor.matmul(c1_ps, c1T_sb[:, kc, fc * 128:(fc + 1) * 128],
                                         hbf[:, kc, gsl],
                                         start=(kc == 0), stop=(kc == FC - 1))
                    if flags["c1b"]:
                        nc.scalar.activation(xcls[:, fc, :], c1_ps, AF.Relu,
                                             bias=bias_sb[:, bcol["c1b"] + fc:bcol["c1b"] + fc + 1])
                    else:
                        nc.scalar.activation(xcls[:, fc, :], c1_ps, AF.Relu)
                outsb = wk.tile([128, KC, C], F32, bufs=2, name="outsb")
                for tc4 in range(KC):
                    c2_ps = ps.tile([128, C], F32, tag="mm", bufs=4, name="c2_ps")
                    for ic in range(FC):
                        nc.tensor.matmul(c2_ps, xcls[:, ic, tc4 * 128:(tc4 + 1) * 128],
                                         c2T_sb[:, ic, :],
                                         start=(ic == 0), stop=(ic == FC - 1 and not flags["c2b"]))
                    if flags["c2b"]:
                        nc.tensor.matmul(c2_ps, ones128r, c2b_sb, start=False, stop=True)
                    nc.vector.tensor_copy(outsb[:, tc4, :], c2_ps)
                for tc4 in range(KC):
                    nc.sync.dma_start(out=d_out[g * N + tc4 * 128:g * N + (tc4 + 1) * 128, :],
                                      in_=outsb[:, tc4, :])

    nc.compile()
    return nc


def _row_of(nc, wk, bias_sb, bcol, name, l, fc, n):
    """Materialize a [1, n] bf16 row from a per-partition bias column via ...

    Not used when biases are zero; build a broadcastable row by copying the
    bias column (128 partitions) is wrong-axis, so this path requires a
    host-provided row tensor instead. Kept unreachable for zero biases."""
    raise NotImplementedError("non-zero out_b/lin2_b not supported in this build")


def _layernorm(nc, tc, wk, ps, flags, bias_sb, bcol, which, l, g,
               h_sb, hbf, invn, epsw, onesrow, ones128f):
    """LayerNorm over features (partitions) for graph g, in place on h_sb,
    writing the bf16 cast into hbf. h_sb currently holds the pre-LN u."""
    N_ = N
    gsl = slice(g * N_, (g + 1) * N_)
    # sums over features via PE; eps folded into the sum-of-squares chain
    ms_ps = ps.tile([1, N_], F32, tag="sc", bufs=2, name="ms_ps")
    sq_ps = ps.tile([1, N_], F32, tag="sc", bufs=2, name="sq_ps")
    for fc in range(FC):
        nc.tensor.matmul(ms_ps, invn, hbf[:, fc, gsl],
                         start=(fc == 0), stop=(fc == FC - 1))
    for fc in range(FC):
        u2 = wk.tile([128, N_], BF16, bufs=2, name="u2")
        nc.scalar.activation(u2, h_sb[:, fc, gsl], AF.Square)
        nc.tensor.matmul(sq_ps, invn, u2, start=(fc == 0), stop=False)
    nc.tensor.matmul(sq_ps, epsw, onesrow, start=False, stop=True)
    # row math packed into one [128, N] tile: p0=m, p32=m^2/var
    rows = wk.tile([128, N_], F32, bufs=2, name="rows")
    nc.scalar.activation(rows[0:1, :], ms_ps, AF.Copy)
    nc.vector.tensor_mul(rows[32:33, :], rows[0:1, :], rows[0:1, :])
    nc.vector.tensor_sub(rows[32:33, :], sq_ps, rows[32:33, :])
    # broadcast m and var across partitions with K=1 fp32 outer products
    MB_ps = ps.tile([128, N_], F32, tag="sc", bufs=2, name="MB_ps")
    VB_ps = ps.tile([128, N_], F32, tag="sc", bufs=2, name="VB_ps")
    nc.tensor.matmul(MB_ps, ones128f[0:1, :], rows[0:1, :], start=True, stop=True)
    nc.tensor.matmul(VB_ps, ones128f[32:33, :], rows[32:33, :], start=True, stop=True)
    # wide 1/var (custom DVE, SBUF source) then wide sqrt -> rB in SBUF
    VB_sb = wk.tile([128, N_], F32, bufs=1, name="VB_sb")
    nc.scalar.activation(VB_sb, VB_ps, AF.Copy)
    RVB = wk.tile([128, N_], F32, bufs=1, name="RVB")
    nc.vector.reciprocal_approx_fast(out=RVB, in_=VB_sb)
    rB = wk.tile([128, N_], F32, bufs=2, name="rB")
    nc.scalar.activation(rB, RVB, AF.Sqrt)
    # apply: h = (u - MB) * rB  (optionally * s + b)
    s_on = flags[which + "s"]
    b_on = flags[which + "b"]
    for fc in range(FC):
        tt = wk.tile([128, N_], F32, bufs=2, name="tt")
        nc.vector.tensor_sub(tt, h_sb[:, fc, gsl], MB_ps)
        if not s_on and not b_on:
            nc.vector.tensor_mul(h_sb[:, fc, gsl], tt, rB)
        else:
            nc.vector.tensor_mul(tt, tt, rB)
            sc = bias_sb[:, bcol[which + "s"] + l * 4 + fc:bcol[which + "s"] + l * 4 + fc + 1] \
                if s_on else 1.0
            bi = bias_sb[:, bcol[which + "b"] + l * 4 + fc:bcol[which + "b"] + l * 4 + fc + 1] \
                if b_on else 0.0
            nc.scalar.activation(h_sb[:, fc, gsl], tt, AF.Copy, bias=bi, scale=sc)


_CACHE = {}


def _prep_inputs(inputs):
    """Host-side preprocessing -> (flags, per-core in_maps)."""
    x = np.asarray(inputs["x"], np.float32)
    pe = np.asarray(inputs["pe"], np.float32)
    degree = np.asarray(inputs["degree"], np.float32)
    W_emb = np.asarray(inputs["W_emb"], np.float32)
    qkv_w = np.asarray(inputs["qkv_w"], np.float32)
    qkv_b = np.asarray(inputs["qkv_b"], np.float32)
    out_w = np.asarray(inputs["out_w"], np.float32)
    out_b = np.asarray(inputs["out_b"], np.float32)
    lin1_w = np.asarray(inputs["lin1_w"], np.float32)
    lin1_b = np.asarray(inputs["lin1_b"], np.float32)
    lin2_w = np.asarray(inputs["lin2_w"], np.float32)
    lin2_b = np.asarray(inputs["lin2_b"], np.float32)
    ln1_s = np.asarray(inputs["ln1_s"], np.float32)
    ln1_b = np.asarray(inputs["ln1_b"], np.float32)
    ln2_s = np.asarray(inputs["ln2_s"], np.float32)
    ln2_b = np.asarray(inputs["ln2_b"], np.float32)
    cls1_w = np.asarray(inputs["cls1_w"], np.float32)
    cls1_b = np.asarray(inputs["cls1_b"], np.float32)
    cls2_w = np.asarray(inputs["cls2_w"], np.float32)
    cls2_b = np.asarray(inputs["cls2_b"], np.float32)

    flags = {
        "qkvb": bool(np.any(qkv_b[:, :2 * D])),
        "vb": bool(np.any(qkv_b[:, 2 * D:])),
        "ob": bool(np.any(out_b)),
        "l1b": bool(np.any(lin1_b)),
        "l2b": bool(np.any(lin2_b)),
        "c1b": bool(np.any(cls1_b)),
        "c2b": bool(np.any(cls2_b)),
        "ln1s": bool(np.any(ln1_s != 1.0)),
        "ln1b": bool(np.any(ln1_b)),
        "ln2s": bool(np.any(ln2_s != 1.0)),
        "ln2b": bool(np.any(ln2_b)),
    }
    if flags["ob"] or flags["l2b"]:
        raise NotImplementedError("non-zero out_b/lin2_b not supported")

    # bias blob layout
    cols = {}
    ncols = 0

    def add(name, width):
        nonlocal ncols
        cols[name] = ncols
        ncols += width

    blob_parts = []
    if flags["qkvb"]:
        add("qkvb", L * 8)
        qb = qkv_b[:, :2 * D].reshape(L, 8, 128)
        qb = qb.copy()
        qb[:, :4, :] *= 0.125  # q-bias carries the 1/sqrt(dh) fold
        blob_parts.append(qb.transpose(2, 0, 1).reshape(128, L * 8))
    if flags["l1b"]:
        add("l1b", L * 16)
        blob_parts.append(lin1_b.reshape(L, 16, 128).transpose(2, 0, 1).reshape(128, L * 16))
    if flags["c1b"]:
        add("c1b", 4)
        blob_parts.append(cls1_b.reshape(4, 128).T)
    for nm, arr in (("ln1s", ln1_s), ("ln1b", ln1_b), ("ln2s", ln2_s), ("ln2b", ln2_b)):
        if flags[nm]:
            add(nm, L * 4)
            blob_parts.append(arr.reshape(L, 4, 128).transpose(2, 0, 1).reshape(128, L * 4))
    flags["bias_cols"] = cols
    flags["nbias_cols"] = ncols
    blob = np.concatenate(blob_parts, axis=1).astype(np.float32) if blob_parts else None

    qkvT = np.ascontiguousarray(qkv_w.transpose(0, 2, 1)).copy()
    qkvT[:, :, :D] *= 0.125
    shared = {
        "wembT": np.ascontiguousarray(W_emb.T).astype(bf),
        "qkvT": qkvT.astype(bf),
        "owT": np.ascontiguousarray(out_w.transpose(0, 2, 1)).astype(bf),
        "l1T": np.ascontiguousarray(lin1_w.transpose(0, 2, 1)).astype(bf),
        "l2T": np.ascontiguousarray(lin2_w.transpose(0, 2, 1)).astype(bf),
        "c1T": np.ascontiguousarray(cls1_w.T).astype(bf),
        "c2T": np.ascontiguousarray(cls2_w.T).astype(bf),
    }
    if blob is not None:
        shared["biasblob"] = blob
    if flags["vb"]:
        shared["vbias"] = qkv_b[:, 2 * D:].reshape(L, 1, D).astype(bf)
    if flags["c2b"]:
        shared["c2bias"] = cls2_b.reshape(1, C).astype(bf)

    peb = (pe + 1e-6).transpose(0, 2, 1)
    xT = x.transpose(0, 2, 1)
    in_maps = []
    for c in range(NCORES):
        gs = slice(c * G, (c + 1) * G)
        im = dict(shared)
        im["xT"] = np.ascontiguousarray(xT[gs]).astype(bf)
        im["peT"] = np.ascontiguousarray(peb[gs]).astype(bf)
        im["deg"] = np.ascontiguousarray(degree[gs]).astype(np.float32)
        in_maps.append(im)
    return flags, in_maps


def kernel(**inputs):
    flags, in_maps = _prep_inputs(inputs)
    key = tuple(sorted((k, v) for k, v in flags.items() if k not in ("bias_cols",)
                       and not isinstance(v, dict)))
    if key not in _CACHE:
        _CACHE[key] = _build(flags)
    nc = _CACHE[key]
    res = bass_utils.run_bass_kernel_spmd(nc, in_maps, list(range(NCORES)))
    out = np.concatenate([r["out"] for r in res.results], axis=0)
    return out.astype(np.float32)


def run_traced(inputs, tmpdir="/tmp/ntff_out"):
    """For test.py: run with NTFF tracing, return (output, results)."""
    import shutil
    flags, in_maps = _prep_inputs(inputs)
    nc = _build(flags)
    if tmpdir:
        shutil.rmtree(tmpdir, ignore_errors=True)
        os.makedirs(tmpdir, exist_ok=True)
    res = bass_utils.run_bass_kernel_spmd(nc, in_maps, list(range(NCORES)),
                                          trace=True, tmpdir=tmpdir)
    out = np.concatenate([r["out"] for r in res.results], axis=0)
    return out.astype(np.float32), res
